# revision 1
# baseline (speedup 1.0000x reference)
"""CliffordAttention TRN2 kernel.

Math: the reference's orientation-bias einsum chain is folded into the Wq
projection. With A_h[i,j] = scale*rev[j]*met[j]*delta_ij + rev[i]*wk[h,i,j]
(wk = biv_kernel . bias_weight), the total pre-softmax logits are exactly
q_eff . k where q_eff = x @ Wq_eff.T and Wq_eff mixes blades of Wq by A_h.
bk shifts every logit in a softmax row equally (drops out exactly); bv adds
bv to the output exactly (softmax rows sum to 1) and is applied on host.

Sharding: tensor-parallel over heads — core c computes heads 2c, 2c+1 for
all batches. x (feature-major, bf16) is replicated; W slices per core.
Device kernel per core: q/k projections in feature-major layout (qT/kT),
v in row-major, then per (b, head): S.T = k q^T (m on partitions), P=exp(S.T)
(no max subtraction needed: |logits| < ~20), out.T = v^T P accumulated on PE,
denominators via ones-vector matmuls. Host divides/transposes/concats.
"""

import os

import numpy as np
import ml_dtypes

BF16 = ml_dtypes.bfloat16

# Problem shapes (hardcoded per contract).
B = 4
L = 1024
NH = 16
CD = 32
NB = 8
DM = NH * CD * NB  # 4096
DH = CD * NB  # 256 head dim
N_CORES = 8
HPC = NH // N_CORES  # heads per core
FPC = HPC * DH  # 512 output features per core
ROWS = B * L  # 4096
SIG_G = (1.0, 1.0, 1.0)
BIV_IDX = (4, 5, 6)


def _build_tables():
    """Geometric-product structure constants; copied from the model spec."""
    g = SIG_G
    dim = len(g)
    n = 2**dim
    masks = sorted(range(n), key=lambda m: (bin(m).count("1"), m))
    inv = {m: i for i, m in enumerate(masks)}
    C = np.zeros((n, n, n), dtype=np.float32)
    for i, a in enumerate(masks):
        for j, b in enumerate(masks):
            sw, at = 0, a >> 1
            while at:
                sw += bin(at & b).count("1")
                at >>= 1
            s = -1.0 if (sw & 1) else 1.0
            for t in range(dim):
                if (a >> t) & 1 and (b >> t) & 1:
                    s *= float(g[t])
            C[i, j, inv[a ^ b]] = s
    grades = np.array([bin(m).count("1") for m in masks])
    rev = np.where((grades * (grades - 1) // 2) % 2 == 1, -1.0, 1.0).astype(np.float32)
    met = np.array(
        [
            float(np.prod([g[t] for t in range(dim) if (m >> t) & 1] + [1.0]))
            for m in masks
        ],
        dtype=np.float32,
    )
    return C, rev, met


_C_NP, _REV_NP, _MET_NP = _build_tables()

# Device-kernel tiling constants.
KT = DM // 128  # 32 contraction tiles
NFT = FPC // 128  # 4 output-feature tiles per core
N_RQ = ROWS // 1024  # 4 row quads
N_MT = L // 128  # 8 key tiles per (b, h)
N_LB = L // 512  # 2 query blocks per (b, h)
N_KF = DH // 128  # 2 feature tiles per head

_CACHE = {}


def _build_nc():
    import concourse.tile as tile
    from concourse import bacc, mybir

    f32 = mybir.dt.float32
    bf16 = mybir.dt.bfloat16
    Exp = mybir.ActivationFunctionType.Exp
    CopyF = mybir.ActivationFunctionType.Copy
    IdentF = mybir.ActivationFunctionType.Identity

    nc = bacc.Bacc(
        "TRN2",
        target_bir_lowering=False,
        debug=False,
        enable_asserts=False,
        num_devices=N_CORES,
    )

    xT_d = nc.dram_tensor("xT", [DM, ROWS], bf16, kind="ExternalInput").ap()
    wqT_d = nc.dram_tensor("wqT", [DM, FPC], bf16, kind="ExternalInput").ap()
    wkT_d = nc.dram_tensor("wkT", [DM, FPC], bf16, kind="ExternalInput").ap()
    wvT_d = nc.dram_tensor("wvT", [DM, FPC], bf16, kind="ExternalInput").ap()
    bqe_d = nc.dram_tensor("bqe", [FPC], f32, kind="ExternalInput").ap()
    outT_d = nc.dram_tensor("outT", [FPC, ROWS], f32, kind="ExternalOutput").ap()
    # denominator partial sums: summed over the partition axis on host
    den_d = nc.dram_tensor(
        "den", [HPC * B, 128, L], f32, kind="ExternalOutput"
    ).ap()

    with tile.TileContext(nc) as tc:
        with tc.tile_pool(name="persist", bufs=1) as persist:
            qT = [
                persist.tile([128, ROWS], bf16, tag=f"qT{f}", name=f"qT{f}")
                for f in range(NFT)
            ]
            kT = [
                persist.tile([128, ROWS], bf16, tag=f"kT{f}", name=f"kT{f}")
                for f in range(NFT)
            ]
            vv = persist.tile([128, ROWS // 128, FPC], bf16, tag="vv", name="vv")
            bq_sb = persist.tile([128, NFT], f32, tag="bq", name="bq_sb")

            # ---- projections ----
            # Weights live in per-k-tile tiles (32 tags, bufs=2) so the first
            # matmul of a phase only waits on one small DMA, and the next
            # phase's weights prefetch during the current phase's compute.
            with (
                tc.tile_pool(name="wpool", bufs=2) as wp,
                tc.tile_pool(name="xs", bufs=10) as xp,
                tc.tile_pool(name="psp", bufs=1, space="PSUM") as pp,
            ):

                def load_w(w_dram, tagp):
                    # weights ride the Activation-issued HWDGE queues so the
                    # x stream (Sync queues) has no head-of-line blocking
                    w4 = w_dram.rearrange("(kt p) f -> p kt f", p=128)
                    tiles = []
                    for kt in range(KT):
                        wt = wp.tile([128, FPC], bf16, tag=f"w{kt}", name=f"w{tagp}{kt}")
                        nc.scalar.dma_start(out=wt, in_=w4[:, kt, :])
                        tiles.append(wt)
                    return tiles

                # interleave the q/k weight loads: the fused projection needs
                # wq[kt] AND wk[kt] together, and the HWDGE queue is in-order
                wq4 = wqT_d.rearrange("(kt p) f -> p kt f", p=128)
                wk4 = wkT_d.rearrange("(kt p) f -> p kt f", p=128)
                wq_t, wk_t = [], []
                for kt in range(KT):
                    wqt = wp.tile([128, FPC], bf16, tag=f"w{kt}", name=f"wq{kt}")
                    nc.scalar.dma_start(out=wqt, in_=wq4[:, kt, :])
                    wq_t.append(wqt)
                    wkt = wp.tile([128, FPC], bf16, tag=f"w{kt}", name=f"wk{kt}")
                    nc.scalar.dma_start(out=wkt, in_=wk4[:, kt, :])
                    wk_t.append(wkt)
                nc.scalar.dma_start(
                    out=bq_sb, in_=bqe_d.rearrange("(f p) -> p f", p=128)
                )

                def load_x(rq, kt, tagp):
                    xt = xp.tile([128, 1024], bf16, tag="x", name=f"x{tagp}t")
                    nc.sync.dma_start(
                        out=xt,
                        in_=xT_d[
                            kt * 128 : (kt + 1) * 128, rq * 1024 : (rq + 1) * 1024
                        ],
                    )
                    return xt

                # fused q+k projection: one pass over x feeds both weight
                # sets (halves x traffic where cross-core HBM contention is
                # worst); psum = 4 q banks + 4 k banks per 512-row block
                wv_t = None
                for rblk in range(ROWS // 512):
                    if rblk == 3:
                        # delay the v-weight prefetch past the startup burst
                        # (all 8 cores hammer HBM in the first ~50us)
                        wv_t = load_w(wvT_d, "v")
                    pq = [
                        pp.tile([128, 512], f32, tag=f"ps{i}", name=f"psq{i}")
                        for i in range(NFT)
                    ]
                    pk = [
                        pp.tile([128, 512], f32, tag=f"ps{NFT + i}", name=f"psk{i}")
                        for i in range(NFT)
                    ]
                    for kt in range(KT):
                        xt = xp.tile([128, 512], bf16, tag="x", name="xqkt")
                        nc.sync.dma_start(
                            out=xt,
                            in_=xT_d[
                                kt * 128 : (kt + 1) * 128,
                                rblk * 512 : (rblk + 1) * 512,
                            ],
                        )
                        for f in range(NFT):
                            nc.tensor.matmul(
                                pq[f][:],
                                lhsT=wq_t[kt][:, f * 128 : (f + 1) * 128],
                                rhs=xt[:],
                                start=(kt == 0),
                                stop=(kt == KT - 1),
                            )
                        for f in range(NFT):
                            nc.tensor.matmul(
                                pk[f][:],
                                lhsT=wk_t[kt][:, f * 128 : (f + 1) * 128],
                                rhs=xt[:],
                                start=(kt == 0),
                                stop=(kt == KT - 1),
                            )
                    # evacuate on both Vector and Scalar so the PSUM banks
                    # free ~2x faster at block boundaries
                    for f in range(NFT):
                        dst = qT[f][:, rblk * 512 : (rblk + 1) * 512]
                        if f % 2 == 0:
                            nc.vector.tensor_scalar_add(
                                dst, pq[f][:], bq_sb[:, f : f + 1]
                            )
                        else:
                            nc.scalar.activation(
                                dst, pq[f][:], IdentF, bias=bq_sb[:, f : f + 1]
                            )
                    for f in range(NFT):
                        dst = kT[f][:, rblk * 512 : (rblk + 1) * 512]
                        if f % 2 == 0:
                            nc.vector.tensor_copy(dst, pk[f][:])
                        else:
                            nc.scalar.activation(dst, pk[f][:], CopyF)

                # v projection: out[row_tile, f] = xT.T @ wvT (row-major v)
                for rq in range(N_RQ):
                    pst = [
                        pp.tile([128, FPC], f32, tag=f"ps{i}", name=f"psv{i}")
                        for i in range(8)
                    ]
                    for kt in range(KT):
                        xt = load_x(rq, kt, "v")
                        for rt in range(8):
                            nc.tensor.matmul(
                                pst[rt][:],
                                lhsT=xt[:, rt * 128 : (rt + 1) * 128],
                                rhs=wv_t[kt][:],
                                start=(kt == 0),
                                stop=(kt == KT - 1),
                            )
                    for rt in range(8):
                        if rt % 2 == 0:
                            nc.vector.tensor_copy(vv[:, rq * 8 + rt, :], pst[rt][:])
                        else:
                            nc.scalar.activation(vv[:, rq * 8 + rt, :], pst[rt][:], CopyF)

            # ---- attention per (b, local head) ----
            with (
                tc.tile_pool(name="stp", bufs=4, space="PSUM") as stp,
                tc.tile_pool(name="otp", bufs=1, space="PSUM") as otp,
                tc.tile_pool(name="ptp", bufs=24) as ptp,
                tc.tile_pool(name="osb", bufs=6) as osbp,
                tc.tile_pool(name="dtr", bufs=10) as dtrp,
            ):
                for b in range(B):
                    for hl in range(HPC):
                        ots = [
                            otp.tile([128, 512], f32, tag=f"ot{i}", name=f"ot{i}")
                            for i in range(N_KF * N_LB)
                        ]
                        pts = {}
                        dacc = {}
                        for mt in range(N_MT):
                            for lb in range(N_LB):
                                st = stp.tile([128, 512], f32, tag="st", name="st")
                                for kf in range(N_KF):
                                    nc.tensor.matmul(
                                        st[:],
                                        lhsT=kT[HPC * hl + kf][
                                            :, b * L + mt * 128 : b * L + (mt + 1) * 128
                                        ],
                                        rhs=qT[HPC * hl + kf][
                                            :, b * L + lb * 512 : b * L + (lb + 1) * 512
                                        ],
                                        start=(kf == 0),
                                        stop=(kf == N_KF - 1),
                                    )
                                pt = ptp.tile([128, 512], bf16, tag="pt", name="pt")
                                nc.scalar.activation(pt[:], st[:], Exp)
                                pts[(mt, lb)] = pt
                                # running denominator partial sum (keeps the
                                # final reduction off the kernel tail)
                                if mt == 1:
                                    s = dtrp.tile([128, 512], f32, tag="dtree", name="dts")
                                    nc.vector.tensor_add(
                                        s[:], pts[(0, lb)][:], pt[:]
                                    )
                                    dacc[lb] = s
                                elif mt > 1:
                                    s = dtrp.tile([128, 512], f32, tag="dtree", name="dts")
                                    nc.vector.tensor_add(s[:], dacc[lb][:], pt[:])
                                    dacc[lb] = s
                            for vf in range(N_KF):
                                for lb in range(N_LB):
                                    nc.tensor.matmul(
                                        ots[vf * N_LB + lb][:],
                                        lhsT=vv[
                                            :,
                                            b * 8 + mt,
                                            hl * DH + vf * 128 : hl * DH + (vf + 1) * 128,
                                        ],
                                        rhs=pts[(mt, lb)][:],
                                        start=(mt == 0),
                                        stop=(mt == N_MT - 1),
                                    )
                        for vf in range(N_KF):
                            for lb in range(N_LB):
                                ot_sb = osbp.tile([128, 512], f32, tag="osb", name="ot_sb")
                                nc.vector.tensor_copy(ot_sb, ots[vf * N_LB + lb][:])
                                nc.sync.dma_start(
                                    out=outT_d[
                                        hl * DH + vf * 128 : hl * DH + (vf + 1) * 128,
                                        b * L + lb * 512 : b * L + (lb + 1) * 512,
                                    ],
                                    in_=ot_sb,
                                )
                        for lb in range(N_LB):
                            nc.sync.dma_start(
                                out=den_d[hl * B + b, :, lb * 512 : (lb + 1) * 512],
                                in_=dacc[lb][:],
                            )

    nc.compile()
    return nc


def _get_nc():
    if "nc" not in _CACHE:
        _CACHE["nc"] = _build_nc()
    return _CACHE["nc"]


def kernel(x, Wq, bq, Wk, bk, Wv, bv, bias_weight):
    from concourse.bass_utils import run_bass_kernel_spmd

    x = np.asarray(x, dtype=np.float32)
    Wq = np.asarray(Wq, dtype=np.float32)
    Wk = np.asarray(Wk, dtype=np.float32)
    Wv = np.asarray(Wv, dtype=np.float32)
    bq = np.asarray(bq, dtype=np.float32)
    bv = np.asarray(bv, dtype=np.float32)
    bias_weight = np.asarray(bias_weight, dtype=np.float32)

    # Fold the orientation bias + scale + blade reverse/metric into Wq.
    scale = 1.0 / np.sqrt(CD * NB)
    bivC = _C_NP[:, :, list(BIV_IDX)]  # [NB, NB, 3]
    wk_mix = np.einsum("ijc,hc->hij", bivC, bias_weight)  # [NH, NB, NB]
    A = _REV_NP[None, :, None] * wk_mix + scale * np.diag(_REV_NP * _MET_NP)[None]
    # Wq_eff[(h,d,j), f] = sum_i A[h,i,j] * Wq[(h,d,i), f]
    Wq4 = Wq.reshape(NH, CD, NB, DM)
    Wq_eff = np.matmul(Wq4.transpose(0, 1, 3, 2), A[:, None]).transpose(0, 1, 3, 2)
    Wq_eff = np.ascontiguousarray(Wq_eff).reshape(DM, DM)
    bq_eff = np.matmul(bq.reshape(NH, CD, NB)[:, :, None, :], A[:, None])
    bq_eff = bq_eff.reshape(DM).astype(np.float32)

    xT = np.ascontiguousarray(x.reshape(ROWS, DM).T).astype(BF16)

    nc = _get_nc()
    in_maps = []
    for c in range(N_CORES):
        sl = slice(c * FPC, (c + 1) * FPC)
        in_maps.append(
            {
                "xT": xT,
                "wqT": np.ascontiguousarray(Wq_eff[sl].T).astype(BF16),
                "wkT": np.ascontiguousarray(Wk[sl].T).astype(BF16),
                "wvT": np.ascontiguousarray(Wv[sl].T).astype(BF16),
                "bqe": np.ascontiguousarray(bq_eff[sl]),
            }
        )

    res = run_bass_kernel_spmd(
        nc,
        in_maps,
        core_ids=list(range(N_CORES)),
        trace=bool(int(os.environ.get("KERNEL_TRACE", "0"))),
    )
    _CACHE["last_results"] = res

    # Gather: out[b, l, c*FPC + hl*DH + f] = outT_c[hl*DH+f, b*L+l] / den_c[hl*B+b, l]
    parts = []
    for c in range(N_CORES):
        outT = res.results[c]["outT"].reshape(HPC, DH, B, L)
        den = res.results[c]["den"].sum(axis=1).reshape(HPC, B, L)
        part = outT.transpose(2, 3, 0, 1) / den.transpose(1, 2, 0)[:, :, :, None]
        parts.append(part.reshape(B, L, FPC))
    out = np.concatenate(parts, axis=2)
    out += bv[None, None, :]
    return out.astype(np.float32)



# revision 2
# speedup vs baseline: 1.1954x; 1.1954x over previous
"""CliffordAttention TRN2 kernel.

Math: the reference's orientation-bias einsum chain is folded into the Wq
projection. With A_h[i,j] = scale*rev[j]*met[j]*delta_ij + rev[i]*wk[h,i,j]
(wk = biv_kernel . bias_weight), the total pre-softmax logits are exactly
q_eff . k where q_eff = x @ Wq_eff.T and Wq_eff mixes blades of Wq by A_h.
bk shifts every logit in a softmax row equally (drops out exactly); bv adds
bv to the output exactly (softmax rows sum to 1) and is applied on host.

Sharding: tensor-parallel over heads — core c computes heads 2c, 2c+1 for
all batches. x (feature-major, bf16) is replicated; W slices per core.
Device kernel per core: q/k projections in feature-major layout (qT/kT),
v in row-major, then per (b, head): S.T = k q^T (m on partitions), P=exp(S.T)
(no max subtraction needed: |logits| < ~20), out.T = v^T P accumulated on PE,
denominators via ones-vector matmuls. Host divides/transposes/concats.
"""

import os

import numpy as np
import ml_dtypes

BF16 = ml_dtypes.bfloat16

# Problem shapes (hardcoded per contract).
B = 4
L = 1024
NH = 16
CD = 32
NB = 8
DM = NH * CD * NB  # 4096
DH = CD * NB  # 256 head dim
N_CORES = 8
HPC = NH // N_CORES  # heads per core
FPC = HPC * DH  # 512 output features per core
ROWS = B * L  # 4096
SIG_G = (1.0, 1.0, 1.0)
BIV_IDX = (4, 5, 6)


def _build_tables():
    """Geometric-product structure constants; copied from the model spec."""
    g = SIG_G
    dim = len(g)
    n = 2**dim
    masks = sorted(range(n), key=lambda m: (bin(m).count("1"), m))
    inv = {m: i for i, m in enumerate(masks)}
    C = np.zeros((n, n, n), dtype=np.float32)
    for i, a in enumerate(masks):
        for j, b in enumerate(masks):
            sw, at = 0, a >> 1
            while at:
                sw += bin(at & b).count("1")
                at >>= 1
            s = -1.0 if (sw & 1) else 1.0
            for t in range(dim):
                if (a >> t) & 1 and (b >> t) & 1:
                    s *= float(g[t])
            C[i, j, inv[a ^ b]] = s
    grades = np.array([bin(m).count("1") for m in masks])
    rev = np.where((grades * (grades - 1) // 2) % 2 == 1, -1.0, 1.0).astype(np.float32)
    met = np.array(
        [
            float(np.prod([g[t] for t in range(dim) if (m >> t) & 1] + [1.0]))
            for m in masks
        ],
        dtype=np.float32,
    )
    return C, rev, met


_C_NP, _REV_NP, _MET_NP = _build_tables()

# Device-kernel tiling constants.
KT = DM // 128  # 32 contraction tiles
NFT = FPC // 128  # 4 output-feature tiles per core
N_RQ = ROWS // 1024  # 4 row quads
N_MT = L // 128  # 8 key tiles per (b, h)
N_LB = L // 512  # 2 query blocks per (b, h)
N_KF = DH // 128  # 2 feature tiles per head

_CACHE = {}


def _build_nc():
    import concourse.tile as tile
    from concourse import bacc, mybir

    f32 = mybir.dt.float32
    bf16 = mybir.dt.bfloat16
    Exp = mybir.ActivationFunctionType.Exp
    CopyF = mybir.ActivationFunctionType.Copy
    IdentF = mybir.ActivationFunctionType.Identity

    nc = bacc.Bacc(
        "TRN2",
        target_bir_lowering=False,
        debug=False,
        enable_asserts=False,
        num_devices=N_CORES,
    )

    xT_d = nc.dram_tensor("xT", [DM, ROWS], bf16, kind="ExternalInput").ap()
    wqT_d = nc.dram_tensor("wqT", [DM, FPC], bf16, kind="ExternalInput").ap()
    wkT_d = nc.dram_tensor("wkT", [DM, FPC], bf16, kind="ExternalInput").ap()
    wvT_d = nc.dram_tensor("wvT", [DM, FPC], bf16, kind="ExternalInput").ap()
    bqe_d = nc.dram_tensor("bqe", [FPC], f32, kind="ExternalInput").ap()
    outT_d = nc.dram_tensor("outT", [FPC, ROWS], f32, kind="ExternalOutput").ap()
    # denominator partial sums: summed over the partition axis on host
    den_d = nc.dram_tensor(
        "den", [HPC * B, 128, L], f32, kind="ExternalOutput"
    ).ap()

    with tile.TileContext(nc) as tc:
        with tc.tile_pool(name="persist", bufs=1) as persist:
            qT = [
                persist.tile([128, ROWS], bf16, tag=f"qT{f}", name=f"qT{f}")
                for f in range(NFT)
            ]
            kT = [
                persist.tile([128, ROWS], bf16, tag=f"kT{f}", name=f"kT{f}")
                for f in range(NFT)
            ]
            vv = persist.tile([128, ROWS // 128, FPC], bf16, tag="vv", name="vv")
            bq_sb = persist.tile([128, NFT], f32, tag="bq", name="bq_sb")

            # ---- projections ----
            # Weights live in per-k-tile tiles (32 tags, bufs=2) so the first
            # matmul of a phase only waits on one small DMA, and the next
            # phase's weights prefetch during the current phase's compute.
            with (
                tc.tile_pool(name="wpool", bufs=2) as wp,
                tc.tile_pool(name="xs", bufs=10) as xp,
                tc.tile_pool(name="psp", bufs=1, space="PSUM") as pp,
            ):

                def load_w(w_dram, tagp):
                    # weights ride the Activation-issued HWDGE queues so the
                    # x stream (Sync queues) has no head-of-line blocking
                    w4 = w_dram.rearrange("(kt p) f -> p kt f", p=128)
                    tiles = []
                    for kt in range(KT):
                        wt = wp.tile([128, FPC], bf16, tag=f"w{kt}", name=f"w{tagp}{kt}")
                        nc.scalar.dma_start(out=wt, in_=w4[:, kt, :])
                        tiles.append(wt)
                    return tiles

                # interleave the q/k weight loads: the fused projection needs
                # wq[kt] AND wk[kt] together, and the HWDGE queue is in-order
                wq4 = wqT_d.rearrange("(kt p) f -> p kt f", p=128)
                wk4 = wkT_d.rearrange("(kt p) f -> p kt f", p=128)
                wq_t, wk_t = [], []
                for kt in range(KT):
                    wqt = wp.tile([128, FPC], bf16, tag=f"w{kt}", name=f"wq{kt}")
                    nc.scalar.dma_start(out=wqt, in_=wq4[:, kt, :])
                    wq_t.append(wqt)
                    wkt = wp.tile([128, FPC], bf16, tag=f"w{kt}", name=f"wk{kt}")
                    nc.scalar.dma_start(out=wkt, in_=wk4[:, kt, :])
                    wk_t.append(wkt)
                nc.scalar.dma_start(
                    out=bq_sb, in_=bqe_d.rearrange("(f p) -> p f", p=128)
                )

                def load_x(rq, kt, tagp):
                    xt = xp.tile([128, 1024], bf16, tag="x", name=f"x{tagp}t")
                    nc.sync.dma_start(
                        out=xt,
                        in_=xT_d[
                            kt * 128 : (kt + 1) * 128, rq * 1024 : (rq + 1) * 1024
                        ],
                    )
                    return xt

                # fused q+k projection: one pass over x feeds both weight
                # sets (halves x traffic where cross-core HBM contention is
                # worst); psum = 4 q banks + 4 k banks per 512-row block
                wv_t = None
                for rblk in range(ROWS // 512):
                    if rblk == 3:
                        # delay the v-weight prefetch past the startup burst
                        # (all 8 cores hammer HBM in the first ~50us)
                        wv_t = load_w(wvT_d, "v")
                    pq = [
                        pp.tile([128, 512], f32, tag=f"ps{i}", name=f"psq{i}")
                        for i in range(NFT)
                    ]
                    pk = [
                        pp.tile([128, 512], f32, tag=f"ps{NFT + i}", name=f"psk{i}")
                        for i in range(NFT)
                    ]
                    for kt in range(KT):
                        xt = xp.tile([128, 512], bf16, tag="x", name="xqkt")
                        nc.sync.dma_start(
                            out=xt,
                            in_=xT_d[
                                kt * 128 : (kt + 1) * 128,
                                rblk * 512 : (rblk + 1) * 512,
                            ],
                        )
                        for f in range(NFT):
                            nc.tensor.matmul(
                                pq[f][:],
                                lhsT=wq_t[kt][:, f * 128 : (f + 1) * 128],
                                rhs=xt[:],
                                start=(kt == 0),
                                stop=(kt == KT - 1),
                            )
                        for f in range(NFT):
                            nc.tensor.matmul(
                                pk[f][:],
                                lhsT=wk_t[kt][:, f * 128 : (f + 1) * 128],
                                rhs=xt[:],
                                start=(kt == 0),
                                stop=(kt == KT - 1),
                            )
                    # evacuate on both Vector and Scalar so the PSUM banks
                    # free ~2x faster at block boundaries
                    for f in range(NFT):
                        dst = qT[f][:, rblk * 512 : (rblk + 1) * 512]
                        if f % 2 == 0:
                            nc.vector.tensor_scalar_add(
                                dst, pq[f][:], bq_sb[:, f : f + 1]
                            )
                        else:
                            nc.scalar.activation(
                                dst, pq[f][:], IdentF, bias=bq_sb[:, f : f + 1]
                            )
                    for f in range(NFT):
                        dst = kT[f][:, rblk * 512 : (rblk + 1) * 512]
                        if f % 2 == 0:
                            nc.vector.tensor_copy(dst, pk[f][:])
                        else:
                            nc.scalar.activation(dst, pk[f][:], CopyF)

                # v projection: out[row_tile, f] = xT.T @ wvT (row-major v)
                for rq in range(N_RQ):
                    pst = [
                        pp.tile([128, FPC], f32, tag=f"ps{i}", name=f"psv{i}")
                        for i in range(8)
                    ]
                    for kt in range(KT):
                        xt = load_x(rq, kt, "v")
                        for rt in range(8):
                            nc.tensor.matmul(
                                pst[rt][:],
                                lhsT=xt[:, rt * 128 : (rt + 1) * 128],
                                rhs=wv_t[kt][:],
                                start=(kt == 0),
                                stop=(kt == KT - 1),
                            )
                    for rt in range(8):
                        if rt % 2 == 0:
                            nc.vector.tensor_copy(vv[:, rq * 8 + rt, :], pst[rt][:])
                        else:
                            nc.scalar.activation(vv[:, rq * 8 + rt, :], pst[rt][:], CopyF)

            # ---- attention per (b, local head) ----
            with (
                tc.tile_pool(name="stp", bufs=4, space="PSUM") as stp,
                tc.tile_pool(name="otp", bufs=1, space="PSUM") as otp,
                tc.tile_pool(name="ptp", bufs=24) as ptp,
                tc.tile_pool(name="osb", bufs=6) as osbp,
                tc.tile_pool(name="dtr", bufs=10) as dtrp,
            ):
                for b in range(B):
                    for hl in range(HPC):
                        ots = [
                            otp.tile([128, 512], f32, tag=f"ot{i}", name=f"ot{i}")
                            for i in range(N_KF * N_LB)
                        ]
                        pts = {}
                        dacc = {}
                        for mt in range(N_MT):
                            for lb in range(N_LB):
                                st = stp.tile([128, 512], f32, tag="st", name="st")
                                for kf in range(N_KF):
                                    nc.tensor.matmul(
                                        st[:],
                                        lhsT=kT[HPC * hl + kf][
                                            :, b * L + mt * 128 : b * L + (mt + 1) * 128
                                        ],
                                        rhs=qT[HPC * hl + kf][
                                            :, b * L + lb * 512 : b * L + (lb + 1) * 512
                                        ],
                                        start=(kf == 0),
                                        stop=(kf == N_KF - 1),
                                    )
                                pt = ptp.tile([128, 512], bf16, tag="pt", name="pt")
                                nc.scalar.activation(pt[:], st[:], Exp)
                                pts[(mt, lb)] = pt
                                # running denominator partial sum (keeps the
                                # final reduction off the kernel tail)
                                if mt == 1:
                                    s = dtrp.tile([128, 512], f32, tag="dtree", name="dts")
                                    nc.vector.tensor_add(
                                        s[:], pts[(0, lb)][:], pt[:]
                                    )
                                    dacc[lb] = s
                                elif mt > 1:
                                    s = dtrp.tile([128, 512], f32, tag="dtree", name="dts")
                                    nc.vector.tensor_add(s[:], dacc[lb][:], pt[:])
                                    dacc[lb] = s
                            for vf in range(N_KF):
                                for lb in range(N_LB):
                                    nc.tensor.matmul(
                                        ots[vf * N_LB + lb][:],
                                        lhsT=vv[
                                            :,
                                            b * 8 + mt,
                                            hl * DH + vf * 128 : hl * DH + (vf + 1) * 128,
                                        ],
                                        rhs=pts[(mt, lb)][:],
                                        start=(mt == 0),
                                        stop=(mt == N_MT - 1),
                                    )
                        for vf in range(N_KF):
                            for lb in range(N_LB):
                                ot_sb = osbp.tile([128, 512], f32, tag="osb", name="ot_sb")
                                nc.vector.tensor_copy(ot_sb, ots[vf * N_LB + lb][:])
                                nc.sync.dma_start(
                                    out=outT_d[
                                        hl * DH + vf * 128 : hl * DH + (vf + 1) * 128,
                                        b * L + lb * 512 : b * L + (lb + 1) * 512,
                                    ],
                                    in_=ot_sb,
                                )
                        for lb in range(N_LB):
                            nc.sync.dma_start(
                                out=den_d[hl * B + b, :, lb * 512 : (lb + 1) * 512],
                                in_=dacc[lb][:],
                            )

    nc.compile()
    return nc


def _get_nc():
    if "nc" not in _CACHE:
        _CACHE["nc"] = _build_nc()
    return _CACHE["nc"]


def kernel(x, Wq, bq, Wk, bk, Wv, bv, bias_weight):
    from concourse.bass_utils import run_bass_kernel_spmd

    x = np.asarray(x, dtype=np.float32)
    Wq = np.asarray(Wq, dtype=np.float32)
    Wk = np.asarray(Wk, dtype=np.float32)
    Wv = np.asarray(Wv, dtype=np.float32)
    bq = np.asarray(bq, dtype=np.float32)
    bv = np.asarray(bv, dtype=np.float32)
    bias_weight = np.asarray(bias_weight, dtype=np.float32)

    # Fold the orientation bias + scale + blade reverse/metric into Wq.
    scale = 1.0 / np.sqrt(CD * NB)
    bivC = _C_NP[:, :, list(BIV_IDX)]  # [NB, NB, 3]
    wk_mix = np.einsum("ijc,hc->hij", bivC, bias_weight)  # [NH, NB, NB]
    A = _REV_NP[None, :, None] * wk_mix + scale * np.diag(_REV_NP * _MET_NP)[None]
    # Wq_eff[(h,d,j), f] = sum_i A[h,i,j] * Wq[(h,d,i), f]
    Wq4 = Wq.reshape(NH, CD, NB, DM)
    Wq_eff = np.matmul(Wq4.transpose(0, 1, 3, 2), A[:, None]).transpose(0, 1, 3, 2)
    Wq_eff = np.ascontiguousarray(Wq_eff).reshape(DM, DM)
    bq_eff = np.matmul(bq.reshape(NH, CD, NB)[:, :, None, :], A[:, None])
    bq_eff = bq_eff.reshape(DM).astype(np.float32)

    xT = np.ascontiguousarray(x.reshape(ROWS, DM).T).astype(BF16)

    nc = _get_nc()
    in_maps = []
    for c in range(N_CORES):
        sl = slice(c * FPC, (c + 1) * FPC)
        in_maps.append(
            {
                # per-core copy: identical arrays get deduped into ONE HBM
                # buffer, and 8 cores streaming the same 32MB region at
                # ~74GB/s each suffer HBM contention that uniformly slows
                # every engine ~1.2x (measured 259ns vs 216ns per matmul)
                "xT": xT if c == 0 else xT.copy(),
                "wqT": np.ascontiguousarray(Wq_eff[sl].T).astype(BF16),
                "wkT": np.ascontiguousarray(Wk[sl].T).astype(BF16),
                "wvT": np.ascontiguousarray(Wv[sl].T).astype(BF16),
                "bqe": np.ascontiguousarray(bq_eff[sl]),
            }
        )

    res = run_bass_kernel_spmd(
        nc,
        in_maps,
        core_ids=list(range(N_CORES)),
        trace=bool(int(os.environ.get("KERNEL_TRACE", "0"))),
    )
    _CACHE["last_results"] = res

    # Gather: out[b, l, c*FPC + hl*DH + f] = outT_c[hl*DH+f, b*L+l] / den_c[hl*B+b, l]
    parts = []
    for c in range(N_CORES):
        outT = res.results[c]["outT"].reshape(HPC, DH, B, L)
        den = res.results[c]["den"].sum(axis=1).reshape(HPC, B, L)
        part = outT.transpose(2, 3, 0, 1) / den.transpose(1, 2, 0)[:, :, :, None]
        parts.append(part.reshape(B, L, FPC))
    out = np.concatenate(parts, axis=2)
    out += bv[None, None, :]
    return out.astype(np.float32)

